# revision 1
# baseline (speedup 1.0000x reference)
"""DGCN layer (message passing GNN) on 8 Trainium2 NeuronCores via Bass/Tile.

Strategy (matches the dst-sharded hint):
  - Nodes are range-partitioned across the 8 cores (6250 nodes/core).
  - Each core owns every edge whose dst lies in its node range, so the
    segment-sum over dst is fully core-local.
  - h is replicated into each core's HBM at input staging time (this plays
    the role of the all-gather of src features); the per-edge random
    feat[src] read is an on-device dma_gather (custom SWDGE ucode), run on
    two SWDGE queues so descriptor generation is parallel across Q7 cores.
  - dma_gather indices are int16 (< 32768), so h is addressed as two
    tables (rows [0, 32768) and [32768, N)); each window's edges are
    grouped into lo-table tiles then hi-table tiles.
  - Per-edge coefficient  coef_e = alpha^dist_e * outdeg[src_e]^-1/2  and the
    per-node output scale  s_v = indeg[v]^-3/2  are tiny O(E)/O(N) host
    scalars computed during sharding; all O(E*D) work runs on device.

Device pipeline per core:
  phase 1 (edge aggregation, accumulates agg^T[feat, node] in SBUF):
    per 128-dst-node window: two dma_gathers (lo/hi tables) fetch all the
    window's h[src] rows; per 128-edge tile sel = (iota == r_e) * coef_e
    (DVE), psum += matmul(lhsT=G_tile, rhs=sel) accumulating over the
    window; copy the finished window column-block into agg^T.
  phase 2: rst[128 nodes, D] = matmul(lhsT=agg^T window, rhs=W),
    * s_v (per-partition broadcast) + bias, DMA out as [nodes, D].
"""

import math

import numpy as np

P = 128
ALPHA = 0.5
N_CORES = 8
SPLIT = 32768  # int16 index limit for dma_gather


def _wrap_idx16(flat):
    """dma_gather index layout: entry k -> partition k%16, column k//16,
    replicated across the 8 gpsimd core groups (partitions 16-127)."""
    n = flat.shape[-1]
    assert n % 16 == 0
    cols = n // 16
    w = np.asarray(flat, np.int16).reshape(cols, 16).T  # [16, cols]
    return np.tile(w, (8, 1))  # [128, cols]


def _prep_host(h, src, dst, distance, n_cores):
    """Shard edges by dst range; build per-core padded tile arrays."""
    N, D = h.shape
    E = src.shape[0]
    npc = N // n_cores
    n_windows = (npc + P - 1) // P

    src = np.asarray(src).astype(np.int64)
    dst = np.asarray(dst).astype(np.int64)
    distance = np.asarray(distance)

    out_deg = np.bincount(src, minlength=N).astype(np.float64)
    in_deg = np.bincount(dst, minlength=N).astype(np.float64)
    coef_all = (np.float64(ALPHA) ** distance.astype(np.float64)) * (
        out_deg[src] ** -0.5
    )
    s_all = in_deg**-1.5  # applied after the W matmul

    # Balanced node -> (core, window, slot) assignment: deal nodes (sorted by
    # in-degree) into the n_cores*n_windows bins in rounds; within a round the
    # heaviest hi-degree nodes go to the lightest bins. This equalizes each
    # window's lo/hi edge counts, minimizing the padded tile count T (which is
    # a global max across bins). The host un-permutes output rows at the end.
    n_bins = n_cores * n_windows
    lo_deg = np.bincount(dst[src < SPLIT], minlength=N).astype(np.int64)
    hi_deg = np.bincount(dst[src >= SPLIT], minlength=N).astype(np.int64)
    order_nodes = np.argsort(-(lo_deg + hi_deg), kind="stable")
    node_bin = np.empty(N, np.int64)
    node_slot = np.empty(N, np.int64)
    lo_sum = np.zeros(n_bins, np.int64)
    hi_sum = np.zeros(n_bins, np.int64)
    fill = np.zeros(n_bins, np.int64)
    pos = 0
    rnd = 0
    while pos < N:
        take = min(n_bins, N - pos)
        nodes_r = order_nodes[pos : pos + take]
        nodes_r = nodes_r[np.argsort(-hi_deg[nodes_r], kind="stable")]
        bins_r = np.argsort(hi_sum, kind="stable")[:take]
        node_bin[nodes_r] = bins_r
        node_slot[nodes_r] = fill[bins_r]
        fill[bins_r] += 1
        lo_sum[bins_r] += lo_deg[nodes_r]
        hi_sum[bins_r] += hi_deg[nodes_r]
        pos += take
        rnd += 1
    node_core = node_bin // n_windows
    node_window = node_bin % n_windows

    core_of = node_core[dst]
    w_of = node_window[dst]
    r_of = node_slot[dst].astype(np.float32)
    is_hi = (src >= SPLIT).astype(np.int64)

    # sort edges by (core, window, lo/hi) — stable
    gw = (core_of * n_windows + w_of) * 2 + is_hi
    n_gw = n_cores * n_windows * 2
    counts = np.bincount(gw, minlength=n_gw)
    cl = counts.reshape(n_cores, n_windows, 2)
    T_lo = max(1, int(math.ceil(cl[:, :, 0].max() / P)))
    T_hi = max(1, int(math.ceil(cl[:, :, 1].max() / P)))
    T = T_lo + T_hi
    n_cols = n_windows * T

    order = np.argsort(gw, kind="stable")
    sgw = gw[order]
    win_start = np.concatenate([[0], np.cumsum(counts)[:-1]])
    q = np.arange(E, dtype=np.int64) - win_start[sgw]  # pos within group

    core_arr = sgw // (2 * n_windows)
    w_arr = (sgw // 2) % n_windows
    hi_arr = sgw % 2
    j_arr = q // P + hi_arr * T_lo  # hi tiles come after the lo tiles
    p_arr = q % P
    col_arr = w_arr * T + j_arr

    rofs = np.zeros((n_cores, P, n_cols), np.float32)
    coef = np.zeros((n_cores, P, n_cols), np.float32)
    rofs[core_arr, p_arr, col_arr] = r_of[order]
    coef[core_arr, p_arr, col_arr] = coef_all[order].astype(np.float32)

    # int16 gather indices, padded with 0 (coef 0 nullifies), table-relative
    srcrel = np.zeros((n_cores, P, n_cols), np.int64)
    srcrel[core_arr, p_arr, col_arr] = src[order] - (src[order] >= SPLIT) * SPLIT

    # wrapped idx16: per core, per window: lo block then hi block.
    # Blocks start at 64B-aligned column offsets (32 int16 cols).
    CL, CH = T_lo * 8, T_hi * 8  # int16 cols per window per table
    CLa = (CL + 31) // 32 * 32
    CHa = (CH + 31) // 32 * 32
    idx16 = np.zeros((n_cores, P, n_windows * (CLa + CHa)), np.int16)
    for c in range(n_cores):
        flat = srcrel[c].T  # [n_cols, P]: (tile, lane)
        for w in range(n_windows):
            lo = flat[w * T : w * T + T_lo].reshape(-1)
            hi = flat[w * T + T_lo : (w + 1) * T].reshape(-1)
            base = w * (CLa + CHa)
            idx16[c, :, base : base + CL] = _wrap_idx16(lo)
            idx16[c, :, base + CLa : base + CLa + CH] = _wrap_idx16(hi)

    snode = np.ones((n_cores, P, n_windows), np.float32)
    snode[node_core, node_slot, node_window] = s_all.astype(np.float32)

    # host-side inverse permutation: node v lives at core_out row
    # node_window*128 + node_slot of core node_core
    out_core = node_core
    out_row = node_window * P + node_slot

    return (
        idx16, rofs, coef, snode, out_core, out_row,
        n_windows, T_lo, T_hi, n_cols,
    )


def _build_nc(N, D, n_windows, T_lo, T_hi, n_cols):
    import concourse.bacc as bacc
    import concourse.tile as tile
    from concourse import mybir

    f32 = mybir.dt.float32
    i16 = mybir.dt.int16
    T = T_lo + T_hi
    CL, CH = T_lo * 8, T_hi * 8
    CLa = (CL + 31) // 32 * 32
    CHa = (CH + 31) // 32 * 32

    # fconst free-dim layout: rofs | coef | iota | wmat | biasf | snode
    ftot = 2 * n_cols + P + D + D + n_windows

    nc = bacc.Bacc(
        None, target_bir_lowering=False, debug=False, num_swdge_queues=2
    )
    h_d = nc.declare_dram_parameter("h", [N, D], f32, isOutput=False)
    idx_d = nc.declare_dram_parameter(
        "idx16", [P, n_windows * (CLa + CHa)], i16, isOutput=False
    )
    fc_d = nc.declare_dram_parameter("fconst", [P, ftot], f32, isOutput=False)
    out_d = nc.declare_dram_parameter("out", [n_windows * P, D], f32, isOutput=True)

    mult = mybir.AluOpType.mult

    with tile.TileContext(nc) as tc:
        with (
            tc.tile_pool(name="singles", bufs=1) as singles,
            tc.tile_pool(name="glo", bufs=3) as glopool,
            tc.tile_pool(name="ghi", bufs=3) as ghipool,
            tc.tile_pool(name="sel", bufs=6) as selpool,
            tc.tile_pool(name="psum", bufs=4, space="PSUM") as psumpool,
            tc.tile_pool(name="psum2", bufs=2, space="PSUM") as psum2pool,
            tc.tile_pool(name="outp", bufs=3) as outpool,
        ):
            idx_sb = singles.tile([P, n_windows * (CLa + CHa)], i16)
            tot = n_windows * (CLa + CHa)
            hd = min(2, n_windows) * (CLa + CHa)
            nc.sync.dma_start(out=idx_sb[:, :hd], in_=idx_d[:, :hd])
            if hd < tot:
                nc.sync.dma_start(out=idx_sb[:, hd:], in_=idx_d[:, hd:])
            fc_sb = singles.tile([P, ftot], f32)
            nc.sync.dma_start(out=fc_sb[:], in_=fc_d[:])

            r_sb = fc_sb[:, 0:n_cols]
            c_sb = fc_sb[:, n_cols : 2 * n_cols]
            o0 = 2 * n_cols
            io_sb = fc_sb[:, o0 : o0 + P]
            w_sb = fc_sb[:, o0 + P : o0 + P + D]
            b_sb = fc_sb[:, o0 + P + D : o0 + P + 2 * D]
            s_sb = fc_sb[:, o0 + P + 2 * D : o0 + P + 2 * D + n_windows]

            agg = singles.tile([P, n_windows * P], f32)  # agg^T [feat, node]

            mid = n_windows - n_windows // 3 if n_windows >= 6 else n_windows

            def _phase2(w2):
                ps2 = psum2pool.tile([P, D], f32)
                nc.tensor.matmul(
                    out=ps2[:],
                    lhsT=agg[:, w2 * P : (w2 + 1) * P],
                    rhs=w_sb,
                    start=True,
                    stop=True,
                )
                o = outpool.tile([P, D], f32)
                nc.vector.tensor_tensor(
                    out=o[:],
                    in0=ps2[:],
                    in1=s_sb[:, w2 : w2 + 1].to_broadcast([P, D]),
                    op=mult,
                )
                nc.vector.tensor_add(out=o[:], in0=o[:], in1=b_sb)
                nc.sync.dma_start(out=out_d[w2 * P : (w2 + 1) * P, :], in_=o[:])

            h_lo = h_d[0 : min(SPLIT, N), :]
            hi_base = SPLIT if N > SPLIT else 0
            h_hi = h_d[hi_base:N, :]

            GCH = 8  # tiles per dma_gather (hw limit: <=1024 idxs/inst)
            qctr = 0
            for w in range(n_windows):
                base = w * (CLa + CHa)
                lo_chunks = []
                for k in range((T_lo + GCH - 1) // GCH):
                    nt = min(GCH, T_lo - k * GCH)
                    g = glopool.tile([P, GCH, P], f32, tag="glo")
                    cb = base + k * GCH * 8
                    nc.gpsimd.dma_gather(
                        g[:, :nt, :],
                        h_lo,
                        idx_sb[:, cb : cb + nt * 8],
                        nt * P,
                        nt * P,
                        P,
                        single_packet=False,
                        queue_num=qctr % 2,
                    )
                    qctr += 1
                    lo_chunks.append(g)
                hi_chunks = []
                for k in range((T_hi + GCH - 1) // GCH):
                    nt = min(GCH, T_hi - k * GCH)
                    g = ghipool.tile([P, GCH, P], f32, tag="ghi")
                    cb = base + CLa + k * GCH * 8
                    nc.gpsimd.dma_gather(
                        g[:, :nt, :],
                        h_hi,
                        idx_sb[:, cb : cb + nt * 8],
                        nt * P,
                        nt * P,
                        P,
                        single_packet=False,
                        queue_num=qctr % 2,
                    )
                    qctr += 1
                    hi_chunks.append(g)
                ps = psumpool.tile([P, P], f32)
                for j in range(T):
                    t = w * T + j
                    sel = selpool.tile([P, P], f32)
                    nc.vector.tensor_tensor(
                        out=sel[:],
                        in0=r_sb[:, t : t + 1].to_broadcast([P, P]),
                        in1=io_sb,
                        op=mybir.AluOpType.is_equal,
                    )
                    nc.vector.tensor_tensor(
                        out=sel[:],
                        in0=sel[:],
                        in1=c_sb[:, t : t + 1].to_broadcast([P, P]),
                        op=mult,
                    )
                    if j < T_lo:
                        lhsT = lo_chunks[j // GCH][:, j % GCH, :]
                    else:
                        jh = j - T_lo
                        lhsT = hi_chunks[jh // GCH][:, jh % GCH, :]
                    nc.tensor.matmul(
                        out=ps[:],
                        lhsT=lhsT,
                        rhs=sel[:],
                        start=(j == 0),
                        stop=(j == T - 1),
                    )
                nc.scalar.copy(out=agg[:, w * P : (w + 1) * P], in_=ps[:])

                if w == mid - 1:
                    # mid-stream burst: finish output for the windows already
                    # aggregated, while gathers for the rest continue
                    for w2 in range(mid):
                        _phase2(w2)
            for w2 in range(mid, n_windows):
                _phase2(w2)

    nc.compile()
    return nc


def kernel(h, src, dst, distance, weight, bias, _trace=False):
    from concourse.bass_utils import run_bass_kernel_spmd

    h = np.ascontiguousarray(np.asarray(h, dtype=np.float32))
    weight = np.ascontiguousarray(np.asarray(weight, dtype=np.float32))
    bias = np.asarray(bias, dtype=np.float32)
    N, D = h.shape

    (
        idx16, rofs, coef, snode, out_core, out_row,
        n_windows, T_lo, T_hi, n_cols,
    ) = _prep_host(h, src, dst, distance, N_CORES)

    iota = np.broadcast_to(np.arange(P, dtype=np.float32)[None, :], (P, P))
    biasf = np.broadcast_to(bias[None, :], (P, D))

    nc = _build_nc(N, D, n_windows, T_lo, T_hi, n_cols)

    in_maps = []
    for c in range(N_CORES):
        fconst = np.concatenate(
            [rofs[c], coef[c], iota, weight, biasf, snode[c]], axis=1
        ).astype(np.float32)
        in_maps.append(
            {
                "h": h,
                "idx16": np.ascontiguousarray(idx16[c]),
                "fconst": np.ascontiguousarray(fconst),
            }
        )

    res = run_bass_kernel_spmd(nc, in_maps, list(range(N_CORES)), trace=_trace)

    stacked = np.stack([res.results[c]["out"] for c in range(N_CORES)])
    out = stacked[out_core, out_row].astype(np.float32)

    if _trace:
        return out, res
    return out



# revision 2
# speedup vs baseline: 1.0014x; 1.0014x over previous
"""DGCN layer (message-passing GNN) on 8 Trainium2 NeuronCores via Bass/Tile.

v2 strategy (identity slotting, bf16, 4 SWDGE queues, self-loop stream):
  - Gather table h_sc[s] = h[s]*outdeg[s]^-1/2 in bf16; the per-edge
    coefficient reduces to alpha^dist in {1,.5,..,2^-5} (exact in bf16).
  - Nodes sorted by (lo',hi') degree and dealt into bins of 128; bins are
    re-sorted and grouped 8-at-a-time into windows so the 8 cores' bins at
    each window rank have near-identical tile depths (ONE program serves
    all cores; ~5% padding).
  - Identity slotting: node v owns column c of its window; its j-th
    lo-edge sits at (lo tile j, partition c).  The per-tile selection
    matrix is diag(coef): ONE fused DVE multiply per window builds all
    T_w selection tiles, sel[p,j,c] = coef[p,j] * I[p,c].
  - Tile 0 of every window is the SELF-LOOP tile: its feature rows are the
    window's own 128 nodes, preloaded once as a contiguous block -- no
    gather needed for 1/16 of all edges.
  - Remaining edges arrive via SWDGE dma_gather (bf16 rows, 256B) from two
    tables (int16 index limit) on 4 SWDGE queues; gather chunks are packed
    across window boundaries.
  - psum[f,c] += G_j^T @ diag(coef_j); phase 2 applies W, indeg^-3/2, bias.
"""

import numpy as np
import ml_dtypes

P = 128
ALPHA = 0.5
N_CORES = 8
SPLIT = 32768
IPG = 8   # tiles (128 idxs each) per dma_gather instruction
NQ = 4    # SWDGE queues


def _wrap_idx16(flat):
    n = flat.shape[-1]
    assert n % 16 == 0
    cols = n // 16
    w = np.asarray(flat, np.int16).reshape(cols, 16).T
    return np.tile(w, (8, 1))


def _prep_host(h, src, dst, distance, n_cores):
    N, D = h.shape
    E = src.shape[0]
    n_windows = (N + n_cores * P - 1) // (n_cores * P)  # 49

    src = np.asarray(src).astype(np.int64)
    dst = np.asarray(dst).astype(np.int64)
    distance = np.asarray(distance).astype(np.int64)

    out_deg = np.bincount(src, minlength=N).astype(np.float64)
    in_deg = np.bincount(dst, minlength=N).astype(np.float64)
    h_sc = (np.asarray(h, np.float64) * (out_deg**-0.5)[:, None]).astype(
        ml_dtypes.bfloat16
    )
    s_all = (in_deg**-1.5).astype(np.float32)

    # one canonical self-loop edge per node (if present) goes in the
    # per-window self tile; all other edges go in the lo/hi gather streams.
    selfish = src == dst
    first_self = np.full(N, E, np.int64)
    np.minimum.at(first_self, dst[selfish], np.flatnonzero(selfish))
    is_self = np.zeros(E, bool)
    is_self[first_self[first_self < E]] = True

    is_hi = src >= SPLIT
    ns = ~is_self
    lo_deg = np.bincount(dst[ns & ~is_hi], minlength=N).astype(np.int64)
    hi_deg = np.bincount(dst[ns & is_hi], minlength=N).astype(np.int64)

    n_bins = n_cores * n_windows
    order = np.lexsort((hi_deg, lo_deg))
    # two-level sort: within each block of 1024 lo-sorted nodes, re-sort by
    # hi_deg so both per-bin maxes stay near their block means
    BLK = 1024
    for b0 in range(0, N, BLK):
        blk = order[b0:b0 + BLK]
        order[b0:b0 + BLK] = blk[np.lexsort((lo_deg[blk], hi_deg[blk]))]
    ranks = np.empty(N, np.int64)
    ranks[order] = np.arange(N)
    node_bin0 = ranks // P
    node_col = ranks % P

    binTlo = np.zeros(n_bins, np.int64)
    binThi = np.zeros(n_bins, np.int64)
    np.maximum.at(binTlo, node_bin0, lo_deg)
    np.maximum.at(binThi, node_bin0, hi_deg)

    # regroup bins: Tlo-sort, then within chunks of 64 re-sort by Thi, so
    # each consecutive-8 group (one window rank) is tight on BOTH dims
    border = np.lexsort((binThi, binTlo))
    CH = 64
    for c0 in range(0, n_bins, CH):
        ch = border[c0:c0 + CH]
        border[c0:c0 + CH] = ch[np.lexsort((binTlo[ch], binThi[ch]))]
    bin_win = np.empty(n_bins, np.int64)
    bin_core = np.empty(n_bins, np.int64)
    bin_win[border] = np.arange(n_bins) // n_cores
    bin_core[border] = np.arange(n_bins) % n_cores

    wTlo = np.zeros(n_windows, np.int64)
    wThi = np.zeros(n_windows, np.int64)
    np.maximum.at(wTlo, bin_win, binTlo)
    np.maximum.at(wThi, bin_win, binThi)

    # relabel windows so processing order (0..48) interleaves small and
    # large tile counts -- smooths the gather load over time
    iw = []
    a, b = 0, n_windows - 1
    while a <= b:
        iw.append(a)
        if b != a:
            iw.append(b)
        a += 1
        b -= 1
    iw = np.array(iw, np.int64)           # k-th processed = old rank iw[k]
    relabel = np.empty(n_windows, np.int64)
    relabel[iw] = np.arange(n_windows)    # old rank -> new id
    bin_win = relabel[bin_win]
    wTlo = wTlo[iw]
    wThi = wThi[iw]

    node_core = bin_core[node_bin0]
    node_win = bin_win[node_bin0]

    wT = 1 + wTlo + wThi  # +1: self tile
    col_base = np.concatenate([[0], np.cumsum(wT)[:-1]])
    ncols = int(wT.sum())
    max_T = int(wT.max())

    # edge placement (non-self edges)
    e_bin = node_bin0[dst]
    e_col = node_col[dst]
    e_win = bin_win[e_bin]
    e_core = bin_core[e_bin]
    key = (e_bin * 2 + is_hi) * P + e_col
    key_ns = key[ns]
    eorder = np.argsort(key_ns, kind="stable")
    skey = key_ns[eorder]
    starts = np.concatenate(
        [[0], np.cumsum(np.bincount(skey, minlength=2 * n_bins * P))[:-1]]
    )
    ej_ns = np.empty(ns.sum(), np.int64)
    ej_ns[eorder] = np.arange(ns.sum()) - starts[skey]
    ej = np.zeros(E, np.int64)
    ej[ns] = ej_ns

    tilecol = np.where(
        is_self,
        col_base[e_win],
        col_base[e_win] + 1 + np.where(is_hi, wTlo[e_win] + ej, ej),
    )

    coef = np.zeros((n_cores, P, ncols), ml_dtypes.bfloat16)
    coef[e_core, e_col, tilecol] = (np.float64(ALPHA) ** distance).astype(
        ml_dtypes.bfloat16
    )
    srcrel = np.zeros((n_cores, P, ncols), np.int64)
    srcrel[e_core[ns], e_col[ns], tilecol[ns]] = src[ns] - is_hi[ns] * SPLIT

    # self-tile feature block: hblk[core][col, win, :] = h_sc[node]
    hblk = np.zeros((n_cores, P, n_windows, D), ml_dtypes.bfloat16)
    hblk[node_core, node_col, node_win] = h_sc

    # per-stream tile -> coef-column lists (lo tiles exclude the self col)
    lo_cols = np.concatenate(
        [np.arange(col_base[w] + 1, col_base[w] + 1 + wTlo[w]) for w in range(n_windows)]
    ).astype(np.int64)
    hi_cols = np.concatenate(
        [np.arange(col_base[w] + 1 + wTlo[w], col_base[w] + wT[w]) for w in range(n_windows)]
    ).astype(np.int64)

    snode = np.ones((n_cores, P, n_windows), np.float32)
    snode[node_core, node_col, node_win] = s_all

    out_core = node_core
    out_row = node_win * P + node_col

    return (
        h_sc, coef, srcrel, hblk, snode, lo_cols, hi_cols, out_core, out_row,
        wTlo, wThi, n_windows, ncols, max_T,
    )


def _chunk_cols(n_tiles, ipg, col0):
    """Chunk plan [(tile0, ntiles, idx_col_offset)]; 32-col (64B) aligned."""
    plan = []
    off = col0
    for t0 in range(0, n_tiles, ipg):
        nt = min(ipg, n_tiles - t0)
        plan.append((t0, nt, off))
        off += (nt * 8 + 31) // 32 * 32
    return plan, off


def _build_nc(N, D, n_windows, wTlo, wThi, ncols, max_T, idx_cols, ipg, nq):
    import concourse.bacc as bacc
    import concourse.tile as tile
    from concourse import mybir
    from concourse import hw_specs

    # the Tile scheduler paces instructions using this cost model; the
    # stock SWDGE figure (0.34 ns/desc) is ~25x optimistic for this HW's
    # per-queue ucode, which makes the scheduler under-provision gather
    # lead time.  Use the measured figure while building OUR kernel only.
    _swdge_ns = hw_specs.TRN2Spec.SWDGE_NS_PER_DESCRIPTOR

    f32 = mybir.dt.float32
    bf16 = mybir.dt.bfloat16
    i16 = mybir.dt.int16
    mult = mybir.AluOpType.mult

    irep_cols = max_T * P
    n_lo_tiles = int(sum(wTlo))
    n_hi_tiles = int(sum(wThi))
    lo_plan, off = _chunk_cols(n_lo_tiles, ipg, 0)
    hi_plan, off2 = _chunk_cols(n_hi_tiles, ipg, off)
    assert off2 == idx_cols, (off2, idx_cols)

    hw_specs.TRN2Spec.SWDGE_NS_PER_DESCRIPTOR = 8.6 / nq
    nc = bacc.Bacc(None, target_bir_lowering=False, debug=False, num_swdge_queues=nq)
    h_d = nc.declare_dram_parameter("hsc", [N, D], bf16, isOutput=False)
    hblk_d = nc.declare_dram_parameter("hblk", [P, n_windows * D], bf16, isOutput=False)
    idx_d = nc.declare_dram_parameter("idx16", [P, idx_cols], i16, isOutput=False)
    fc16_d = nc.declare_dram_parameter(
        "fc16", [P, ncols + irep_cols + D], bf16, isOutput=False
    )
    fc32_d = nc.declare_dram_parameter("fc32", [P, n_windows], f32, isOutput=False)
    out_d = nc.declare_dram_parameter("out", [n_windows * P, D], f32, isOutput=True)

    h_lo = h_d[0:SPLIT, :]
    h_hi = h_d[SPLIT:N, :]

    with tile.TileContext(nc) as tc:
        with (
            tc.tile_pool(name="singles", bufs=1) as singles,
            tc.tile_pool(name="glo", bufs=36) as glopool,
            tc.tile_pool(name="ghi", bufs=18) as ghipool,
            tc.tile_pool(name="sel", bufs=2) as selpool,
            tc.tile_pool(name="psum", bufs=6, space="PSUM") as psumpool,
            tc.tile_pool(name="psum2", bufs=2, space="PSUM") as psum2pool,
            tc.tile_pool(name="outp", bufs=3) as outpool,
        ):
            idx_sb = singles.tile([P, idx_cols], i16)
            hd = min(idx_cols, 8 * ipg * 8)
            nc.sync.dma_start(out=idx_sb[:, :hd], in_=idx_d[:, :hd])
            if hd < idx_cols:
                nc.sync.dma_start(out=idx_sb[:, hd:], in_=idx_d[:, hd:])
            fc16_sb = singles.tile([P, ncols + irep_cols + D], bf16)
            nc.sync.dma_start(out=fc16_sb[:], in_=fc16_d[:])
            fc32_sb = singles.tile([P, n_windows], f32)
            nc.sync.dma_start(out=fc32_sb[:], in_=fc32_d[:])
            hblk_sb = singles.tile([P, n_windows, D], bf16)
            nc.sync.dma_start(
                out=hblk_sb[:, :, :],
                in_=hblk_d[:, :].rearrange("p (w d) -> p w d", w=n_windows),
            )

            c_sb = fc16_sb[:, 0:ncols]
            irep = fc16_sb[:, ncols:ncols + irep_cols]
            w_sb = fc16_sb[:, ncols + irep_cols:ncols + irep_cols + D]
            s_sb = fc32_sb[:, 0:n_windows]

            agg = singles.tile([P, n_windows * P], bf16)

            lo_tiles = {}
            hi_tiles = {}
            state = {"lo": 0, "hi": 0, "q": 0}

            def emit(stream, plan, pool, src_ap, tiles, tile_needed):
                while state[stream] < len(plan):
                    t0, nt, coff = plan[state[stream]]
                    if t0 > tile_needed:
                        return
                    g = pool.tile([P, ipg, D], bf16, tag="g" + stream)
                    nc.gpsimd.dma_gather(
                        g[:, :nt, :], src_ap, idx_sb[:, coff:coff + nt * 8],
                        nt * P, nt * P, D,
                        single_packet=False, queue_num=state["q"] % nq,
                    )
                    state["q"] += 1
                    for k in range(nt):
                        tiles[t0 + k] = (g, k)
                    state[stream] += 1

            def _phase2(w2):
                ps2 = psum2pool.tile([P, D], f32)
                nc.tensor.matmul(
                    out=ps2[:], lhsT=agg[:, w2 * P:(w2 + 1) * P], rhs=w_sb,
                    start=True, stop=True,
                )
                o = outpool.tile([P, D], f32)
                nc.scalar.mul(out=o[:], in_=ps2[:], mul=s_sb[:, w2:w2 + 1])
                nc.sync.dma_start(out=out_d[w2 * P:(w2 + 1) * P, :], in_=o[:])

            col = 0
            lo_pos = 0
            hi_pos = 0
            for w in range(n_windows):
                Tl, Th = int(wTlo[w]), int(wThi[w])
                T = 1 + Tl + Th
                # emit gathers up to ~8 windows ahead, interleaving each
                # window's lo and hi chunks so a window's last-needed tile
                # is generated promptly
                cl, ch = lo_pos, hi_pos
                for k in range(w, min(w + 9, n_windows)):
                    cl += int(wTlo[k])
                    ch += int(wThi[k])
                    emit("lo", lo_plan, glopool, h_lo, lo_tiles, cl - 1)
                    emit("hi", hi_plan, ghipool, h_hi, hi_tiles, ch - 1)

                sel = selpool.tile([P, max_T, P], bf16, tag="sel")
                nc.vector.tensor_tensor(
                    out=sel[:, :T, :],
                    in0=c_sb[:, col:col + T].to_broadcast([P, T, P]),
                    in1=irep[:, :T * P].rearrange("p (t c) -> p t c", t=T),
                    op=mult,
                )
                ps = psumpool.tile([P, P], f32)
                nc.tensor.matmul(
                    out=ps[:], lhsT=hblk_sb[:, w, :], rhs=sel[:, 0, :],
                    start=True, stop=(T == 1),
                )
                for j in range(1, T):
                    if j - 1 < Tl:
                        g, k = lo_tiles.pop(lo_pos + j - 1)
                    else:
                        g, k = hi_tiles.pop(hi_pos + j - 1 - Tl)
                    nc.tensor.matmul(
                        out=ps[:], lhsT=g[:, k, :], rhs=sel[:, j, :],
                        start=False, stop=(j == T - 1),
                    )
                nc.scalar.copy(out=agg[:, w * P:(w + 1) * P], in_=ps[:])
                col += T
                lo_pos += Tl
                hi_pos += Th
                if w >= 3:
                    _phase2(w - 3)
            for w2 in range(n_windows - 3, n_windows):
                _phase2(w2)

    hw_specs.TRN2Spec.SWDGE_NS_PER_DESCRIPTOR = _swdge_ns
    nc.compile()
    return nc


def kernel(h, src, dst, distance, weight, bias, _trace=False):
    from concourse.bass_utils import run_bass_kernel_spmd

    h = np.ascontiguousarray(np.asarray(h, dtype=np.float32))
    weight = np.asarray(weight, dtype=np.float32)
    bias = np.asarray(bias, dtype=np.float32)
    N, D = h.shape

    (
        h_sc, coef, srcrel, hblk, snode, lo_cols, hi_cols, out_core, out_row,
        wTlo, wThi, n_windows, ncols, max_T,
    ) = _prep_host(h, src, dst, distance, N_CORES)

    n_lo_tiles = len(lo_cols)
    n_hi_tiles = len(hi_cols)
    lo_plan, off = _chunk_cols(n_lo_tiles, IPG, 0)
    hi_plan, idx_cols = _chunk_cols(n_hi_tiles, IPG, off)

    in_maps = []
    irep_cols = max_T * P
    eye = np.eye(P, dtype=ml_dtypes.bfloat16)
    irep = np.tile(eye, (1, max_T))
    wmat16 = weight.astype(ml_dtypes.bfloat16)

    for c in range(N_CORES):
        idx16 = np.zeros((P, idx_cols), np.int16)
        flat = srcrel[c]
        for (t0, nt, coff), cols in (
            [(pl, lo_cols[pl[0]:pl[0] + pl[1]]) for pl in lo_plan]
            + [(pl, hi_cols[pl[0]:pl[0] + pl[1]]) for pl in hi_plan]
        ):
            sub = flat[:, cols].T.reshape(-1)
            idx16[:, coff:coff + nt * 8] = _wrap_idx16(sub)
        fc16 = np.concatenate([coef[c], irep, wmat16], axis=1)
        fc32 = snode[c]
        in_maps.append(
            {
                "hsc": np.ascontiguousarray(h_sc),
                "hblk": np.ascontiguousarray(
                    hblk[c].reshape(P, n_windows * 128)
                ),
                "idx16": idx16,
                "fc16": np.ascontiguousarray(fc16),
                "fc32": np.ascontiguousarray(fc32),
            }
        )

    nc = _build_nc(N, 128, n_windows, wTlo, wThi, ncols, max_T, idx_cols, IPG, NQ)

    res = run_bass_kernel_spmd(nc, in_maps, list(range(N_CORES)), trace=_trace)

    stacked = np.stack([res.results[c]["out"] for c in range(N_CORES)])
    out = stacked[out_core, out_row].astype(np.float32) + bias[None, :]

    if _trace:
        return out, res
    return out
